# revision 33
# baseline (speedup 1.0000x reference)
"""Trainium2 Bass kernel for AttnApply (sliding-window weighted sum).

out[b, t, c] = sum_i padded[b, t+i, c] * weights[b, t, i]   (T=11, D=5 zero pad)

Strategy
--------
Pure data parallel over batch: 8 cores x 4 batches each.

Per core, the windowed sum is a banded matrix multiply on the TensorEngine.
For a time block of M=118 output rows starting at tb (K = M+T-1 = 128):

    out[tb+m, c] = sum_k band[k, m] * in_pad[tb+k, c],   k in [0, 128)

with band[k, m] = w[tb+m, k-m] for 0 <= k-m < T (zero elsewhere); the input
is host zero-padded so edge blocks need no special casing.  Band matrices
are built host-side (cheap scatter of the small weights tensor).

The INPUT tile is the stationary operand and the band the moving operand,
producing the TRANSPOSED output in PSUM:

    psum[c, m] = sum_k in_pad[tb+k, c] * band[k, m]

so PSUM partitions are channels (two 128-channel halves) and the free dim is
time.  Channel-major output means each partition's store is a long contiguous
run in a [C, L] DRAM tensor (host un-transposes at the end) — measured ~5x
faster than time-major stores, which degrade to sub-1KB-per-descriptor
writes on the DRAM side.

Three cost axes drive the design (all verified on this part):
 1. every DMA instruction costs ~600ns of serial HWDGE descriptor-generation
    plus ~600ns of issuing-sequencer time -> use FEW DMA instructions AND
    spread their issue across all three descriptor-generation paths (SP
    HWDGE ring, ACT HWDGE ring, Pool SWDGE);
 2. descriptors must cover multi-KB CONTIGUOUS DRAM runs -> every transfer
    below is partition-major with fat contiguous rows;
 3. program order fixes the DMA service order -> ALL loads are emitted
    before any compute (the whole 13.4MB read set fits in SBUF), so the
    last batch's inputs land ~20us before the store tail and the DMA
    engines run gapless from first descriptor to last.

Layout per supertile of J=7 blocks (5 supertiles per batch; block g covers
out rows [118g, 118g+118)):
 - ONE input load: the input is host-repacked block-major k-major
   [NSUP, K, J*C] (partition k holds row k of each of the supertile's
   blocks side by side; the 10 overlap rows between consecutive blocks are
   duplicated, +8% bytes), so the load is [128, 3584B] over a contiguous
   459KB region.
 - ONE band load [128, (j m)] bf16 = [128, 1652B] over contiguous 211KB.
 - 14 matmuls (7 blocks x 2 channel halves, single bf16 pass) into psum
   [128, J*128] (block stride padded 118->128 for bank alignment),
   compact+convert-copied (f32->bf16) into a per-batch output tile
   [128, LOUT] — ch0 by VectorE, ch1 by ScalarE.
Per batch:
 - TWO stores (one per channel half) [128, 8192B] = 1MB fully contiguous
   (the last batch stores in quarters so the tail drain is shorter).

=> 54 DMA instructions per core and 21.8 MB of HBM traffic — the kernel
sits on the ~360 GB/s per-core DMA bandwidth roofline (TimelineSim shows
the DMA engines gapless from first descriptor to last).

Precision: memory-bound, so everything is single bf16 (input, band, stored
output; host upcasts to f32).  End-to-end relative error 2.8e-3 vs the fp32
reference — well inside the 2e-2 gate — for ~2x less HBM traffic than an
fp32-accurate hi/lo-split variant.  fp8 variants were checked exactly and
exceed the gate (2.7e-2), so bf16 is the floor.
"""

import ml_dtypes
import numpy as np

import concourse.bass as bass  # noqa: F401  (engine handles hang off nc)
import concourse.mybir as mybir
import concourse.tile as tile
from concourse import bacc
from concourse.bass_utils import run_bass_kernel_spmd

B, L, C, T = 32, 4096, 256, 11
D = T // 2
N_CORES = 8
B_LOC = B // N_CORES            # 4 batches per core
M = 118                         # output rows per matmul block
K = M + T - 1                   # 128 = contraction rows per block
NBLK = -(-L // M)               # 35 blocks per batch
J = 7                           # blocks per psum supertile
NSUP = NBLK // J                # 5 supertiles per batch
SUP = M * J                     # 826 output rows per supertile
MP = 128                        # padded per-block psum stride (bank aligned)
LPAD = (NBLK - 1) * M + K       # 4140 padded input rows
LOUT = NBLK * M                 # 4130 output cols in SBUF (34 pad, not stored)

_CACHE: dict = {}
LAST_RESULT = None  # BassKernelResults of the most recent run (for test.py)


def _build_nc(repeat: int = 1, bench: bool = False):
    """Build the bass program. `repeat` re-runs the whole body N times via a
    hardware loop and `bench=True` uses internal zero-filled DRAM
    inputs/outputs with only a tiny external "tick" output — both used only
    for benchmarking; the grading path uses repeat=1, bench=False."""
    nc = bacc.Bacc(
        "TRN2",
        target_bir_lowering=False,
        debug=False,
        num_devices=N_CORES,
    )
    kind_in = "Internal" if bench else "ExternalInput"
    kind_out = "Internal" if bench else "ExternalOutput"
    inp = nc.dram_tensor(
        "in_blk", [B_LOC, NSUP, K, J * C], mybir.dt.bfloat16, kind=kind_in
    ).ap()
    band = nc.dram_tensor(
        "band", [B_LOC, NSUP, K, J * M], mybir.dt.bfloat16, kind=kind_in
    ).ap()
    outT = nc.dram_tensor(
        "outT", [B_LOC, C, L], mybir.dt.bfloat16, kind=kind_out
    ).ap()
    tick = (
        nc.dram_tensor(
            "tick", [1, C], mybir.dt.bfloat16, kind="ExternalOutput"
        ).ap()
        if bench
        else None
    )

    with tile.TileContext(nc) as tc:
        with (
            tc.tile_pool(name="inp", bufs=B_LOC * NSUP) as in_pool,
            tc.tile_pool(name="bnd", bufs=B_LOC * NSUP) as bd_pool,
            tc.tile_pool(name="outp", bufs=4) as o_pool,
            tc.tile_pool(name="ps", bufs=4, space="PSUM") as ps_pool,
        ):
            if bench:
                # back every DRAM page with zeros once per run so reads are
                # real HBM traffic (unbacked-page reads measure absurdly
                # fast and would not represent the grading path)
                with tc.tile_pool(name="z", bufs=1) as z_pool:
                    z = z_pool.tile([K, L], mybir.dt.float32, tag="z")
                    nc.gpsimd.memset(z[:, :], 0.0)
                    zb = z[:, :].bitcast(mybir.dt.bfloat16)  # [128, 4*L]
                    for b in range(B_LOC):
                        for s in range(NSUP):
                            nc.sync.dma_start(out=inp[b, s], in_=zb[:, : J * C])
                            nc.sync.dma_start(out=band[b, s], in_=zb[:, : J * M])
                        for ch in range(2):
                            nc.sync.dma_start(
                                out=outT[b, ch * 128 : (ch + 1) * 128, :],
                                in_=zb[:, :L],
                            )

            def _body():
                # ---- ALL loads first (program order == DMA service
                # order): the entire 13.4MB read set streams before any
                # store competes for DMA bandwidth, so the last batch's
                # compute finishes ~20us before the store tail and the
                # kernel ends right behind the final write ----
                tiles = {}
                for b in range(B_LOC):
                    for s in range(NSUP):
                        in_t = in_pool.tile([K, J * C], mybir.dt.bfloat16, tag="in")
                        ieng = nc.sync if (b * NSUP + s) % 2 == 0 else nc.scalar
                        ieng.dma_start(out=in_t[:, :], in_=inp[b, s])
                        bd_t = bd_pool.tile([K, J * M], mybir.dt.bfloat16, tag="bd")
                        # band loads go through the otherwise-idle Pool
                        # engine's SWDGE: a third descriptor-generation path,
                        # so load issue isn't serialized on one sequencer
                        nc.gpsimd.dma_start(out=bd_t[:, :], in_=band[b, s])
                        tiles[b, s] = (in_t, bd_t)
                for b in range(B_LOC):
                    o_t0 = o_pool.tile([128, LOUT], mybir.dt.bfloat16, tag="o0")
                    o_t1 = o_pool.tile([128, LOUT], mybir.dt.bfloat16, tag="o1")
                    o_ts = [o_t0, o_t1]
                    for s in range(NSUP):
                        t0 = s * SUP
                        in_t, bd_t = tiles[b, s]

                        # ---- matmuls: psum[c, m] per channel half ----
                        for ch in range(2):
                            ps = ps_pool.tile(
                                [128, J * MP], mybir.dt.float32, tag="ps"
                            )
                            for jj in range(J):
                                ih = in_t[:, jj * C + ch * 128 : jj * C + (ch + 1) * 128]
                                bh = bd_t[:, jj * M : (jj + 1) * M]
                                nc.tensor.matmul(
                                    ps[:, jj * MP : jj * MP + M],
                                    ih,
                                    bh,
                                    start=True,
                                    stop=True,
                                )
                            # compact+convert into the per-batch output tile
                            src = ps.rearrange("p (j m) -> p j m", j=J)[:, :, :M]
                            dst = o_ts[ch][:, t0 : t0 + SUP].rearrange(
                                "p (j m) -> p j m", j=J
                            )
                            if ch == 0:
                                nc.vector.tensor_copy(out=dst, in_=src)
                            else:
                                nc.scalar.copy(out=dst, in_=src)
                    # ---- stores: fully-contiguous [128, 8KB] per channel
                    # half; the final batch stores per half-batch so the
                    # tail-end drain is shorter ----
                    if b < B_LOC - 1:
                        for ch in range(2):
                            seng = nc.sync if (b + ch) % 2 == 0 else nc.scalar
                            seng.dma_start(
                                out=outT[b, ch * 128 : (ch + 1) * 128, :],
                                in_=o_ts[ch][:, :L],
                            )
                    else:
                        q = L // 4
                        for qi in range(4):
                            for ch in range(2):
                                seng = nc.sync if (qi + ch) % 2 == 0 else nc.scalar
                                seng.dma_start(
                                    out=outT[
                                        b,
                                        ch * 128 : (ch + 1) * 128,
                                        qi * q : (qi + 1) * q,
                                    ],
                                    in_=o_ts[ch][:, qi * q : (qi + 1) * q],
                                )

            if repeat > 1:
                # hardware loop: constant NEFF size for any repeat count
                # (per-iteration all-engine barrier makes each rep behave
                # like a fresh single-shot run — conservative timing)
                with tc.For_i(0, repeat):
                    _body()
            else:
                _body()

            if tick is not None:
                # flush both HWDGE queues: same-queue reads complete only
                # after all prior writes on that queue
                fl = o_pool.tile([2, C], mybir.dt.bfloat16, tag="fl")
                nc.sync.dma_start(out=fl[0:1, :], in_=outT[0, 0:1, 0:C])
                nc.scalar.dma_start(out=fl[1:2, :], in_=outT[0, 128:129, 0:C])
                nc.sync.dma_start(out=tick[:, :], in_=fl[0:1, :])
                nc.sync.dma_start(out=tick[:, 0:C], in_=fl[1:2, :])
    nc.compile()
    return nc


BF16 = ml_dtypes.bfloat16


def _prep_core(x: np.ndarray, w: np.ndarray):
    """x: [B_LOC, L, C] f32, w: [B_LOC, L, T] f32 -> (in_blk, band) bf16."""
    in_pad = np.zeros((B_LOC, LPAD, C), BF16)
    in_pad[:, D : D + L, :] = x.astype(BF16)
    # supertile-major k-major: in_blk[b, s, k, j*C:(j+1)*C] =
    # in_pad[b, (s*J+j)*M + k, :]
    idx = (M * np.arange(NBLK))[:, None] + np.arange(K)[None, :]  # [NBLK, K]
    in_blk = np.ascontiguousarray(
        in_pad[:, idx, :]
        .reshape(B_LOC, NSUP, J, K, C)
        .transpose(0, 1, 3, 2, 4)
    ).reshape(B_LOC, NSUP, K, J * C)
    band_f32 = np.zeros((B_LOC, NBLK, K, M), np.float32)
    jj, mm = np.meshgrid(np.arange(NBLK), np.arange(M), indexing="ij")
    tt = jj * M + mm
    v = tt < L
    jv, mv_, tv = jj[v], mm[v], tt[v]
    for tau in range(T):
        band_f32[:, jv, mv_ + tau, mv_] = w[:, tv, tau]
    # supertile layout [B_LOC, NSUP, K, J*M]
    band = np.ascontiguousarray(
        band_f32.reshape(B_LOC, NSUP, J, K, M).transpose(0, 1, 3, 2, 4)
    ).reshape(B_LOC, NSUP, K, J * M).astype(BF16)
    return in_blk, band


def kernel(inputs: np.ndarray, weights: np.ndarray) -> np.ndarray:
    global LAST_RESULT
    inputs = np.ascontiguousarray(np.asarray(inputs, dtype=np.float32))
    weights = np.ascontiguousarray(np.asarray(weights, dtype=np.float32))
    assert inputs.shape == (B, L, C) and weights.shape == (B, L, T)

    if "nc" not in _CACHE:
        _CACHE["nc"] = _build_nc()
    nc = _CACHE["nc"]

    in_maps = []
    for c in range(N_CORES):
        sl = slice(c * B_LOC, (c + 1) * B_LOC)
        ib, bd = _prep_core(inputs[sl], weights[sl])
        in_maps.append({"in_blk": ib, "band": bd})

    res = run_bass_kernel_spmd(nc, in_maps, core_ids=list(range(N_CORES)))
    LAST_RESULT = res
    # outputs come back channel-major bf16 [B_LOC, C, L]; un-transpose and
    # upcast to f32 on host
    return np.ascontiguousarray(
        np.concatenate(
            [r["outT"].transpose(0, 2, 1) for r in res.results], axis=0
        )
    ).astype(np.float32)


# revision 34
# speedup vs baseline: 1.0297x; 1.0297x over previous
"""Trainium2 Bass kernel for AttnApply (sliding-window weighted sum).

out[b, t, c] = sum_i padded[b, t+i, c] * weights[b, t, i]   (T=11, D=5 zero pad)

Strategy
--------
Pure data parallel over batch: 8 cores x 4 batches each.

Per core, the windowed sum is a banded matrix multiply on the TensorEngine.
For a time block of M=118 output rows starting at tb (K = M+T-1 = 128):

    out[tb+m, c] = sum_k band[k, m] * in_pad[tb+k, c],   k in [0, 128)

with band[k, m] = w[tb+m, k-m] for 0 <= k-m < T (zero elsewhere); the input
is host zero-padded so edge blocks need no special casing.  Band matrices
are built host-side (cheap scatter of the small weights tensor).

The INPUT tile is the stationary operand and the band the moving operand,
producing the TRANSPOSED output in PSUM:

    psum[c, m] = sum_k in_pad[tb+k, c] * band[k, m]

so PSUM partitions are channels (two 128-channel halves) and the free dim is
time.  Channel-major output means each partition's store is a long contiguous
run in a [C, L] DRAM tensor (host un-transposes at the end) — measured ~5x
faster than time-major stores, which degrade to sub-1KB-per-descriptor
writes on the DRAM side.

Three cost axes drive the design (all verified on this part):
 1. every DMA instruction costs ~600ns of serial HWDGE descriptor-generation
    plus ~600ns of issuing-sequencer time -> use FEW DMA instructions AND
    spread their issue across all three descriptor-generation paths (SP
    HWDGE ring, ACT HWDGE ring, Pool SWDGE);
 2. descriptors must cover multi-KB CONTIGUOUS DRAM runs -> every transfer
    below is partition-major with fat contiguous rows;
 3. program order fixes the DMA service order -> ALL loads are emitted
    before any compute (the whole 13.4MB read set fits in SBUF), so the
    last batch's inputs land ~20us before the store tail and the DMA
    engines run gapless from first descriptor to last.

Layout per supertile of J=7 blocks (5 supertiles per batch; block g covers
out rows [118g, 118g+118)):
 - ONE input load: the input is host-repacked block-major k-major
   [NSUP, K, J*C] (partition k holds row k of each of the supertile's
   blocks side by side; the 10 overlap rows between consecutive blocks are
   duplicated, +8% bytes), so the load is [128, 3584B] over a contiguous
   459KB region.
 - ONE band load [128, (j m)] bf16 = [128, 1652B] over contiguous 211KB.
 - 14 matmuls (7 blocks x 2 channel halves, single bf16 pass) into psum
   [128, J*128] (block stride padded 118->128 for bank alignment),
   compact+convert-copied (f32->bf16) into a per-batch output tile
   [128, LOUT] — ch0 by VectorE, ch1 by ScalarE.
Per batch:
 - TWO stores (one per channel half) [128, 8192B] = 1MB fully contiguous,
   issue alternated across the SP/ACT rings (the last batch stores in
   quarters so the tail drain behind the final copies is shorter).
   Finer-grained stores were A/B-tested and regress: issue overhead
   exceeds any read/write overlap they unlock.

=> 54 DMA instructions per core and 21.8 MB of HBM traffic — the kernel
sits on the ~360 GB/s per-core DMA bandwidth roofline (TimelineSim shows
the DMA engines gapless from first descriptor to last).

Precision: memory-bound, so everything is single bf16 (input, band, stored
output; host upcasts to f32).  End-to-end relative error 2.8e-3 vs the fp32
reference — well inside the 2e-2 gate — for ~2x less HBM traffic than an
fp32-accurate hi/lo-split variant.  fp8 variants were checked exactly and
exceed the gate (2.7e-2), so bf16 is the floor.
"""

import ml_dtypes
import numpy as np

import concourse.bass as bass  # noqa: F401  (engine handles hang off nc)
import concourse.mybir as mybir
import concourse.tile as tile
from concourse import bacc
from concourse.bass_utils import run_bass_kernel_spmd

B, L, C, T = 32, 4096, 256, 11
D = T // 2
N_CORES = 8
B_LOC = B // N_CORES            # 4 batches per core
M = 118                         # output rows per matmul block
K = M + T - 1                   # 128 = contraction rows per block
NBLK = -(-L // M)               # 35 blocks per batch
J = 7                           # blocks per psum supertile
NSUP = NBLK // J                # 5 supertiles per batch
SUP = M * J                     # 826 output rows per supertile
MP = 128                        # padded per-block psum stride (bank aligned)
LPAD = (NBLK - 1) * M + K       # 4140 padded input rows
LOUT = NBLK * M                 # 4130 output cols in SBUF (34 pad, not stored)

_CACHE: dict = {}
LAST_RESULT = None  # BassKernelResults of the most recent run (for test.py)


def _build_nc(repeat: int = 1, bench: bool = False):
    """Build the bass program. `repeat` re-runs the whole body N times via a
    hardware loop and `bench=True` uses internal zero-filled DRAM
    inputs/outputs with only a tiny external "tick" output — both used only
    for benchmarking; the grading path uses repeat=1, bench=False."""
    nc = bacc.Bacc(
        "TRN2",
        target_bir_lowering=False,
        debug=False,
        num_devices=N_CORES,
    )
    kind_in = "Internal" if bench else "ExternalInput"
    kind_out = "Internal" if bench else "ExternalOutput"
    inp = nc.dram_tensor(
        "in_blk", [B_LOC, NSUP, K, J * C], mybir.dt.bfloat16, kind=kind_in
    ).ap()
    band = nc.dram_tensor(
        "band", [B_LOC, NSUP, K, J * M], mybir.dt.bfloat16, kind=kind_in
    ).ap()
    outT = nc.dram_tensor(
        "outT", [B_LOC, C, L], mybir.dt.bfloat16, kind=kind_out
    ).ap()
    tick = (
        nc.dram_tensor(
            "tick", [1, C], mybir.dt.bfloat16, kind="ExternalOutput"
        ).ap()
        if bench
        else None
    )

    with tile.TileContext(nc) as tc:
        with (
            tc.tile_pool(name="inp", bufs=B_LOC * NSUP) as in_pool,
            tc.tile_pool(name="bnd", bufs=B_LOC * NSUP) as bd_pool,
            tc.tile_pool(name="outp", bufs=4) as o_pool,
            tc.tile_pool(name="ps", bufs=4, space="PSUM") as ps_pool,
        ):
            if bench:
                # back every DRAM page with zeros once per run so reads are
                # real HBM traffic (unbacked-page reads measure absurdly
                # fast and would not represent the grading path)
                with tc.tile_pool(name="z", bufs=1) as z_pool:
                    z = z_pool.tile([K, L], mybir.dt.float32, tag="z")
                    nc.gpsimd.memset(z[:, :], 0.0)
                    zb = z[:, :].bitcast(mybir.dt.bfloat16)  # [128, 4*L]
                    for b in range(B_LOC):
                        for s in range(NSUP):
                            nc.sync.dma_start(out=inp[b, s], in_=zb[:, : J * C])
                            nc.sync.dma_start(out=band[b, s], in_=zb[:, : J * M])
                        for ch in range(2):
                            nc.sync.dma_start(
                                out=outT[b, ch * 128 : (ch + 1) * 128, :],
                                in_=zb[:, :L],
                            )

            def _body():
                # ---- ALL loads first (program order == DMA service
                # order): the entire 13.4MB read set streams before any
                # store competes for DMA bandwidth, so the last batch's
                # compute finishes ~20us before the store tail and the
                # kernel ends right behind the final write ----
                tiles = {}
                for b in range(B_LOC):
                    for s in range(NSUP):
                        in_t = in_pool.tile([K, J * C], mybir.dt.bfloat16, tag="in")
                        ieng = nc.sync if (b * NSUP + s) % 2 == 0 else nc.scalar
                        ieng.dma_start(out=in_t[:, :], in_=inp[b, s])
                        bd_t = bd_pool.tile([K, J * M], mybir.dt.bfloat16, tag="bd")
                        # band loads go through the otherwise-idle Pool
                        # engine's SWDGE: a third descriptor-generation path,
                        # so load issue isn't serialized on one sequencer
                        nc.gpsimd.dma_start(out=bd_t[:, :], in_=band[b, s])
                        tiles[b, s] = (in_t, bd_t)
                for b in range(B_LOC):
                    o_t0 = o_pool.tile([128, LOUT], mybir.dt.bfloat16, tag="o0")
                    o_t1 = o_pool.tile([128, LOUT], mybir.dt.bfloat16, tag="o1")
                    o_ts = [o_t0, o_t1]
                    for s in range(NSUP):
                        t0 = s * SUP
                        in_t, bd_t = tiles[b, s]

                        # ---- matmuls: psum[c, m] per channel half ----
                        for ch in range(2):
                            ps = ps_pool.tile(
                                [128, J * MP], mybir.dt.float32, tag="ps"
                            )
                            for jj in range(J):
                                ih = in_t[:, jj * C + ch * 128 : jj * C + (ch + 1) * 128]
                                bh = bd_t[:, jj * M : (jj + 1) * M]
                                nc.tensor.matmul(
                                    ps[:, jj * MP : jj * MP + M],
                                    ih,
                                    bh,
                                    start=True,
                                    stop=True,
                                )
                            # compact+convert into the per-batch output tile
                            src = ps.rearrange("p (j m) -> p j m", j=J)[:, :, :M]
                            dst = o_ts[ch][:, t0 : t0 + SUP].rearrange(
                                "p (j m) -> p j m", j=J
                            )
                            if ch == 0:
                                nc.vector.tensor_copy(out=dst, in_=src)
                            else:
                                nc.scalar.copy(out=dst, in_=src)
                    # ---- stores: fully-contiguous [128, 8KB] per channel
                    # half; the final batch stores per half-batch so the
                    # tail-end drain is shorter ----
                    if b < B_LOC - 1:
                        for ch in range(2):
                            seng = nc.sync if (b + ch) % 2 == 0 else nc.scalar
                            seng.dma_start(
                                out=outT[b, ch * 128 : (ch + 1) * 128, :],
                                in_=o_ts[ch][:, :L],
                            )
                    else:
                        q = L // 4
                        for qi in range(4):
                            for ch in range(2):
                                seng = nc.sync if (qi + ch) % 2 == 0 else nc.scalar
                                seng.dma_start(
                                    out=outT[
                                        b,
                                        ch * 128 : (ch + 1) * 128,
                                        qi * q : (qi + 1) * q,
                                    ],
                                    in_=o_ts[ch][:, qi * q : (qi + 1) * q],
                                )

            if repeat > 1:
                # hardware loop: constant NEFF size for any repeat count
                # (per-iteration all-engine barrier makes each rep behave
                # like a fresh single-shot run — conservative timing)
                with tc.For_i(0, repeat):
                    _body()
            else:
                _body()

            if tick is not None:
                # flush both HWDGE queues: same-queue reads complete only
                # after all prior writes on that queue
                fl = o_pool.tile([2, C], mybir.dt.bfloat16, tag="fl")
                nc.sync.dma_start(out=fl[0:1, :], in_=outT[0, 0:1, 0:C])
                nc.scalar.dma_start(out=fl[1:2, :], in_=outT[0, 128:129, 0:C])
                nc.sync.dma_start(out=tick[:, :], in_=fl[0:1, :])
                nc.sync.dma_start(out=tick[:, 0:C], in_=fl[1:2, :])
    nc.compile()
    return nc


BF16 = ml_dtypes.bfloat16


def _prep_core(x: np.ndarray, w: np.ndarray):
    """x: [B_LOC, L, C] f32, w: [B_LOC, L, T] f32 -> (in_blk, band) bf16."""
    in_pad = np.zeros((B_LOC, LPAD, C), BF16)
    in_pad[:, D : D + L, :] = x.astype(BF16)
    # supertile-major k-major: in_blk[b, s, k, j*C:(j+1)*C] =
    # in_pad[b, (s*J+j)*M + k, :]
    idx = (M * np.arange(NBLK))[:, None] + np.arange(K)[None, :]  # [NBLK, K]
    in_blk = np.ascontiguousarray(
        in_pad[:, idx, :]
        .reshape(B_LOC, NSUP, J, K, C)
        .transpose(0, 1, 3, 2, 4)
    ).reshape(B_LOC, NSUP, K, J * C)
    band_f32 = np.zeros((B_LOC, NBLK, K, M), np.float32)
    jj, mm = np.meshgrid(np.arange(NBLK), np.arange(M), indexing="ij")
    tt = jj * M + mm
    v = tt < L
    jv, mv_, tv = jj[v], mm[v], tt[v]
    for tau in range(T):
        band_f32[:, jv, mv_ + tau, mv_] = w[:, tv, tau]
    # supertile layout [B_LOC, NSUP, K, J*M]
    band = np.ascontiguousarray(
        band_f32.reshape(B_LOC, NSUP, J, K, M).transpose(0, 1, 3, 2, 4)
    ).reshape(B_LOC, NSUP, K, J * M).astype(BF16)
    return in_blk, band


def kernel(inputs: np.ndarray, weights: np.ndarray) -> np.ndarray:
    global LAST_RESULT
    inputs = np.ascontiguousarray(np.asarray(inputs, dtype=np.float32))
    weights = np.ascontiguousarray(np.asarray(weights, dtype=np.float32))
    assert inputs.shape == (B, L, C) and weights.shape == (B, L, T)

    if "nc" not in _CACHE:
        _CACHE["nc"] = _build_nc()
    nc = _CACHE["nc"]

    in_maps = []
    for c in range(N_CORES):
        sl = slice(c * B_LOC, (c + 1) * B_LOC)
        ib, bd = _prep_core(inputs[sl], weights[sl])
        in_maps.append({"in_blk": ib, "band": bd})

    res = run_bass_kernel_spmd(nc, in_maps, core_ids=list(range(N_CORES)))
    LAST_RESULT = res
    # outputs come back channel-major bf16 [B_LOC, C, L]; un-transpose and
    # upcast to f32 on host
    return np.ascontiguousarray(
        np.concatenate(
            [r["outT"].transpose(0, 2, 1) for r in res.results], axis=0
        )
    ).astype(np.float32)
